# revision 27
# baseline (speedup 1.0000x reference)
"""Trainium2 Bass kernel for nn_CoeffNet (gnn_message_passing), v5 (final).

Key structural change vs v2/v3: the basis projection outputs RAW
[g0 | gv] (V=4F=256 cols, was 11F=704 path-weight-prescaled copies).
All 16 CG product groups (a0g0, av_j gv_j, a0 gv, av g0, av x gv
cross halves) are scattered into SEPARATE PSUM column groups by the
segment-sum matmul; the per-channel path weights and the group folds
are applied at NODE level after the segment sum (16x less data than
edge level). This shrinks the PSUM->SBUF copy 11F->4F and the
projection matmul 704->256 rows.

Also vs v2:
  - one-hot scatter matrix S built on host, DMA'd per block (no DVE
    is_equal).
  - plain SWDGE gathers (gen_mode=0) and a single full AllGather per
    refinement. (prepare_only+trigger and split-AG variants were tried:
    the SWDGE trigger does not inherit the table-RAW dep from the
    AllGather in this tile version, which silently gathers stale data.)

kernel(**inputs) takes FULL unsharded inputs, returns FULL (N,4,1) f32.
"""

import os
import sys

import numpy as np

for _p in ("/opt/trn_rl_repo", "/root/.axon_site/_ro/trn_rl_repo"):
    if os.path.isdir(_p) and _p not in sys.path:
        sys.path.insert(0, _p)


class CFG:
    N_NODES = 20000
    F = 64
    KB = 32
    R_REF = 3
    R_MAX = 2.5
    N_CORES = 8
    NPC = 2500
    NBLK = 20
    TPB = 16

    @classmethod
    def derived(cls):
        c = cls
        c.NLOC = c.NBLK * 128
        c.NTAB = c.N_CORES * c.NLOC
        c.EPB = c.TPB * 128
        c.NT = c.NBLK * c.TPB
        c.EPAD = c.NT * 128
        c.ROW = 4 * c.F
        c.VCOLS = 4 * c.F           # g0 | gv(3)
        c.PCOLS = 11 * c.F          # c0(F) c123(F) P2(3F) P3(3F) X(3F)
        return c


CFG.derived()


# ---------------------------------------------------------------- host prep
def _host_geometry(coords, dst_idx, src_idx):
    c = CFG
    r = coords[dst_idx] - coords[src_idx]
    d = np.sqrt(np.sum(r * r, axis=-1) + 1e-12).astype(np.float32)
    rhat = (r / d[:, None]).astype(np.float32)
    centers = np.linspace(0.0, c.R_MAX, c.KB, dtype=np.float32)
    width = centers[1] - centers[0]
    rbf = np.exp(-(((d[:, None] - centers) / width) ** 2)).astype(np.float32)
    return rhat, rbf


def _pack_blocks(deg):
    """Assign NPC nodes to NBLK blocks (<=128 nodes, <=EPB edges each).
    Returns slot_of[node] in [0, NLOC)."""
    c = CFG
    order = np.argsort(-deg, kind="stable")
    loads = np.zeros(c.NBLK, np.int64)
    counts = np.zeros(c.NBLK, np.int64)
    slot_of = np.full(c.NPC, -1, np.int64)
    for n in order:
        d = int(deg[n])
        for b in np.argsort(loads, kind="stable"):
            if counts[b] < 128 and loads[b] + d <= c.EPB:
                slot_of[n] = b * 128 + counts[b]
                loads[b] += d
                counts[b] += 1
                break
        else:
            raise RuntimeError("block packing failed; raise TPB")
    return slot_of


def _host_shard(dst_idx, src_idx, rhat, rbf, x0_rows_bf):
    """Per-core: balanced blocks, edge tiles, gather indices, r0 a-input,
    host-built one-hot scatter matrices."""
    c = CFG
    slot_all = np.zeros(c.N_NODES, np.int64)   # global node -> slot in its core
    core_edges = []
    for ci in range(c.N_CORES):
        m = (dst_idx >= ci * c.NPC) & (dst_idx < (ci + 1) * c.NPC)
        dloc = dst_idx[m] - ci * c.NPC
        deg = np.bincount(dloc, minlength=c.NPC)
        slot_of = _pack_blocks(deg)
        slot_all[ci * c.NPC:(ci + 1) * c.NPC] = slot_of
        core_edges.append((m, dloc))

    # global node -> table row ([core, slot] layout, AllGather concat order)
    cores_of = np.arange(c.N_NODES) // c.NPC
    tab_row_of = cores_of * c.NLOC + slot_all

    cores = []
    for ci in range(c.N_CORES):
        m, dloc = core_edges[ci]
        src_c = src_idx[m]
        rhat_c = rhat[m]
        rbf_c = rbf[m]
        eslot = slot_all[ci * c.NPC:(ci + 1) * c.NPC][dloc]  # dst slot per edge
        blk = eslot >> 7

        order = np.argsort(blk, kind="stable")
        blk_s = blk[order]
        src_s = src_c[order]
        eslot_s = eslot[order]
        rhat_s = rhat_c[order]
        rbf_s = rbf_c[order]
        blk_starts = np.searchsorted(blk_s, np.arange(c.NBLK + 1))

        src_pad = np.zeros(c.EPAD, np.int64)
        dslot_pad = np.full(c.EPAD, -1, np.int64)
        rhat_pad = np.zeros((c.EPAD, 3), np.float32)
        rbf_pad = np.zeros((c.EPAD, c.KB), np.float32)
        avalid = np.zeros(c.EPAD, np.bool_)
        for b in range(c.NBLK):
            s, e = blk_starts[b], blk_starts[b + 1]
            n = e - s
            if n > c.EPB:
                raise RuntimeError(f"core {ci} block {b}: {n} > {c.EPB}")
            o = b * c.EPB
            src_pad[o:o + n] = src_s[s:e]
            dslot_pad[o:o + n] = eslot_s[s:e] & 127
            rhat_pad[o:o + n] = rhat_s[s:e]
            rbf_pad[o:o + n] = rbf_s[s:e]
            avalid[o:o + n] = True

        trow = tab_row_of[src_pad].astype(np.int16)
        trow[~avalid] = 0
        srcw16 = np.ascontiguousarray(trow.reshape(c.EPAD // 16, 16).T)
        srcw = np.tile(srcw16, (8, 1))

        # one-hot scatter matrices [NT, 128 lane, 128 slot]
        smat = np.zeros((c.NT, 128, 128), np.float32)
        lanes = np.arange(c.EPAD) % 128
        tiles = np.arange(c.EPAD) // 128
        v = avalid
        smat[tiles[v], lanes[v], dslot_pad[v]] = 1.0

        basis = np.concatenate(
            [rbf_pad,
             rhat_pad[:, 0:1] * rbf_pad,
             rhat_pad[:, 1:2] * rbf_pad,
             rhat_pad[:, 2:3] * rbf_pad], axis=1)
        basisT = np.ascontiguousarray(
            basis.reshape(c.NT, 128, 4 * c.KB).transpose(0, 2, 1))

        # r0 gathered-a in tile layout [NBLK, 128(part=edge%128), TPB, ROW]
        a0 = x0_rows_bf[src_pad]                       # (EPAD, ROW) bf16
        a0[~avalid] = 0
        a0 = np.ascontiguousarray(
            a0.reshape(c.NBLK, c.TPB, 128, c.ROW).transpose(0, 2, 1, 3))

        cores.append(dict(srcw=srcw, smat=smat, basisT=basisT, a0=a0))
    return cores, slot_all


def _host_weights(W_basis):
    """Raw degree-block-diagonal projection weights [R, 4K, 4F]:
    rows 0:K -> g0 (cols 0:F), rows (j+1)K:(j+2)K -> gv_j."""
    c = CFG
    F = c.F
    K = c.KB
    Wx = np.zeros((c.R_REF, 4 * K, c.VCOLS), np.float32)
    for r in range(c.R_REF):
        W = W_basis[r]
        Wx[r, 0:K, 0:F] = W
        for j in range(3):
            Wx[r, (j + 1) * K:(j + 2) * K, (j + 1) * F:(j + 2) * F] = W
    return Wx


def _tensor_dense_np(x, W0, W1, b0):
    s = x[:, 0] @ W0 + b0
    v = np.einsum("nif,fg->nig", x[:, 1:4], W1)
    return np.concatenate([s[:, None], v], axis=1)


# ---------------------------------------------------------------- bass build
_BUILD_CACHE = {}


def build(with_bias):
    key = (bool(with_bias), CFG.N_CORES, CFG.NBLK, CFG.TPB)
    if key in _BUILD_CACHE:
        return _BUILD_CACHE[key]

    import concourse.bacc as bacc
    import concourse.mybir as mybir
    import concourse.tile as tile
    from concourse import library_config
    from concourse.alu_op_type import AluOpType
    dt = mybir.dt
    c = CFG
    F = c.F
    T = c.TPB
    ROW = c.ROW
    V = c.VCOLS
    PC = c.PCOLS

    nc = bacc.Bacc(num_devices=c.N_CORES, target_bir_lowering=False)

    x0f_d = nc.dram_tensor("x0f", [c.NBLK, 128, ROW], dt.float32, kind="ExternalInput")
    a0_d = nc.dram_tensor("a0", [c.NBLK, 128, T * ROW], dt.bfloat16, kind="ExternalInput")
    srcw_d = nc.dram_tensor("srcw", [128, c.EPAD // 16], dt.int16, kind="ExternalInput")
    smat_d = nc.dram_tensor("smat", [c.NT, 128, 128], dt.bfloat16, kind="ExternalInput")
    basis_d = nc.dram_tensor("basisT", [c.NT, 4 * c.KB, 128], dt.bfloat16, kind="ExternalInput")
    wext_d = nc.dram_tensor("wext", [c.R_REF, 4 * c.KB, V], dt.bfloat16, kind="ExternalInput")
    pvec_d = nc.dram_tensor("pvec", [c.R_REF, 5, 128, F], dt.bfloat16, kind="ExternalInput")
    bvec_d = nc.dram_tensor("bvec", [c.R_REF, 128, F], dt.bfloat16, kind="ExternalInput")
    wout_d = nc.dram_tensor("woutrep", [128, ROW], dt.float32, kind="ExternalInput")
    out_d = nc.dram_tensor("out", [c.NBLK, 128, 4], dt.float32, kind="ExternalOutput")

    xtab1_d = nc.dram_tensor("xtab1", [c.NTAB, ROW], dt.bfloat16, addr_space="Shared")
    xtab2_d = nc.dram_tensor("xtab2", [c.NTAB, ROW], dt.bfloat16, addr_space="Shared")
    xslice_d = nc.dram_tensor("xslice", [c.NLOC, ROW], dt.bfloat16)

    tabs = [None, xtab1_d, xtab2_d]

    with tile.TileContext(nc) as tc:
        with (
            tc.tile_pool(name="resident", bufs=1) as res,
            tc.tile_pool(name="apool", bufs=3) as apool,
            tc.tile_pool(name="vpsum", bufs=4, space="PSUM") as vpsum,
            tc.tile_pool(name="spsum", bufs=2, space="PSUM") as spsum,
            tc.tile_pool(name="vbfp", bufs=2) as vbfp,
            tc.tile_pool(name="ppool", bufs=2) as ppool,
            tc.tile_pool(name="scr", bufs=2) as scr,
            tc.tile_pool(name="basp", bufs=2) as basp,
            tc.tile_pool(name="spool", bufs=2) as spool,
        ):
            xf32 = res.tile([128, c.NBLK, ROW], dt.float32)
            srcw = res.tile([128, c.EPAD // 16], dt.int16)
            wext = res.tile([4 * c.KB, c.R_REF, V], dt.bfloat16)
            pvec = res.tile([128, c.R_REF, 5, F], dt.bfloat16)
            bvec = res.tile([128, c.R_REF, F], dt.bfloat16)
            woutr = res.tile([128, ROW], dt.float32)
            outsb = res.tile([128, c.NBLK, 4], dt.float32)

            nc.sync.dma_start(xf32[:], x0f_d[:].rearrange("b p f -> p b f"))
            nc.sync.dma_start(srcw[:], srcw_d[:])
            nc.sync.dma_start(wext[:], wext_d[:].rearrange("r p col -> p r col"))
            nc.sync.dma_start(pvec[:], pvec_d[:].rearrange("r q p f -> p r q f"))
            nc.sync.dma_start(bvec[:], bvec_d[:].rearrange("r p f -> p r f"))
            nc.sync.dma_start(woutr[:], wout_d[:])

            nc.gpsimd.load_library(library_config.mlp)

            TT = nc.vector.tensor_tensor

            for r in range(c.R_REF):
                for b in range(c.NBLK):
                    a = apool.tile([128, T, ROW], dt.bfloat16, tag="a")
                    if r == 0:
                        nc.sync.dma_start(
                            a[:], a0_d[b].rearrange("p (t f) -> p t f", t=T))
                    else:
                        h = T // 2
                        nc.gpsimd.dma_gather(
                            a[:, 0:h, :], tabs[r][:],
                            srcw[:, b * 128:b * 128 + 64],
                            h * 128, h * 128, ROW)
                        nc.gpsimd.dma_gather(
                            a[:, h:T, :], tabs[r][:],
                            srcw[:, b * 128 + 64:(b + 1) * 128],
                            h * 128, h * 128, ROW)
                    bas = basp.tile([4 * c.KB, T, 128], dt.bfloat16, tag="bas")
                    nc.sync.dma_start(
                        bas[:],
                        basis_d[b * T:(b + 1) * T].rearrange("t p e -> p t e"))
                    smt = spool.tile([128, T, 128], dt.bfloat16, tag="smt")
                    nc.sync.dma_start(
                        smt[:],
                        smat_d[b * T:(b + 1) * T].rearrange("t p n -> p t n"))

                    # vbf = [g0 | gv] raw projection
                    vbf = vbfp.tile([128, T, V], dt.bfloat16, tag="vbf")
                    for t in range(T):
                        vps = vpsum.tile([128, V], dt.float32, tag="vps")
                        nc.tensor.matmul(vps[:], bas[:, t, :], wext[:, r, :],
                                         start=True, stop=True)
                        nc.scalar.activation(
                            vbf[:, t, :], vps[:],
                            mybir.ActivationFunctionType.Copy)

                    if with_bias:
                        TT(vbf[:, :, 0:F], vbf[:, :, 0:F],
                           bvec[:, r, :].unsqueeze(1).to_broadcast([128, T, F]),
                           op=AluOpType.add)

                    a0v = a[:, :, 0:F]
                    av = a[:, :, F:ROW]
                    g0 = vbf[:, :, 0:F]
                    gv = vbf[:, :, F:V]
                    # P = [c0: a0 g0 (F) | c123: sum_j av_j gv_j (F)
                    #      | P2: a0*gv (3F) | P3: av*g0 (3F)
                    #      | X_i = av_{i+1}gv_{i+2} - av_{i+2}gv_{i+1} (3F)]
                    # (dot and cross prefolded on DVE to shrink the PE
                    #  scatter from 16F to 11F columns)
                    P = ppool.tile([128, T, PC], dt.bfloat16, tag="P")
                    tp = scr.tile([128, T, 3 * F], dt.bfloat16, tag="tp")
                    uq = scr.tile([128, T, 3 * F], dt.bfloat16, tag="uq")
                    TT(P[:, :, 0:F], a0v, g0, op=AluOpType.mult)
                    TT(tp[:].rearrange("p t (cc f) -> p t cc f", cc=3),
                       av.rearrange("p t (cc f) -> p t cc f", cc=3),
                       gv.rearrange("p t (cc f) -> p t cc f", cc=3),
                       op=AluOpType.mult)
                    TT(P[:, :, F:2 * F], tp[:, :, 0:F], tp[:, :, F:2 * F],
                       op=AluOpType.add)
                    TT(P[:, :, F:2 * F], P[:, :, F:2 * F],
                       tp[:, :, 2 * F:3 * F], op=AluOpType.add)
                    TT(P[:, :, 2 * F:5 * F].rearrange(
                           "p t (cc f) -> p t cc f", cc=3),
                       a0v.unsqueeze(2).to_broadcast([128, T, 3, F]),
                       gv.rearrange("p t (cc f) -> p t cc f", cc=3),
                       op=AluOpType.mult)
                    TT(P[:, :, 5 * F:8 * F].rearrange(
                           "p t (cc f) -> p t cc f", cc=3),
                       av.rearrange("p t (cc f) -> p t cc f", cc=3),
                       g0.unsqueeze(2).to_broadcast([128, T, 3, F]),
                       op=AluOpType.mult)
                    # q_i = av_{i+1} gv_{i+2} into P; u_i = av_{i+2} gv_{i+1}
                    TT(P[:, :, 8 * F:9 * F], a[:, :, 2 * F:3 * F],
                       vbf[:, :, 3 * F:4 * F], op=AluOpType.mult)
                    TT(P[:, :, 9 * F:10 * F], a[:, :, 3 * F:4 * F],
                       vbf[:, :, F:2 * F], op=AluOpType.mult)
                    TT(P[:, :, 10 * F:11 * F], a[:, :, F:2 * F],
                       vbf[:, :, 2 * F:3 * F], op=AluOpType.mult)
                    TT(uq[:, :, 0:F], a[:, :, 3 * F:4 * F],
                       vbf[:, :, 2 * F:3 * F], op=AluOpType.mult)
                    TT(uq[:, :, F:2 * F], a[:, :, F:2 * F],
                       vbf[:, :, 3 * F:4 * F], op=AluOpType.mult)
                    TT(uq[:, :, 2 * F:3 * F], a[:, :, 2 * F:3 * F],
                       vbf[:, :, F:2 * F], op=AluOpType.mult)
                    TT(P[:, :, 8 * F:11 * F], P[:, :, 8 * F:11 * F], uq[:],
                       op=AluOpType.subtract)

                    psB = spsum.tile([128, PC], dt.float32, tag="psB")
                    for t in range(T):
                        nc.tensor.matmul(
                            psB[:, 0:8 * F], smt[:, t, :], P[:, t, 0:8 * F],
                            start=(t == 0), stop=(t == T - 1))
                    for t in range(T):
                        nc.tensor.matmul(
                            psB[:, 8 * F:PC], smt[:, t, :], P[:, t, 8 * F:PC],
                            start=(t == 0), stop=(t == T - 1))

                    # node-level fold with path weights:
                    # m0 += p0*c0 + p1*c123; mv += p2*cP2 + p3*cP3 + p4*cX
                    # DVE may read only one PSUM operand per op, so stage the
                    # segment sums through a bf16 SBUF copy first (Act).
                    psC = scr.tile([128, PC], dt.bfloat16, tag="psC")
                    nc.scalar.activation(psC[:], psB[:],
                                         mybir.ActivationFunctionType.Copy)
                    tF = scr.tile([128, F], dt.float32, tag="tF")
                    t3 = scr.tile([128, 3 * F], dt.float32, tag="t3")
                    TT(tF[:], psC[:, 0:F], pvec[:, r, 0, :],
                       op=AluOpType.mult)
                    TT(xf32[:, b, 0:F], xf32[:, b, 0:F], tF[:],
                       op=AluOpType.add)
                    TT(tF[:], psC[:, F:2 * F], pvec[:, r, 1, :],
                       op=AluOpType.mult)
                    TT(xf32[:, b, 0:F], xf32[:, b, 0:F], tF[:],
                       op=AluOpType.add)
                    TT(t3[:].rearrange("p (cc f) -> p cc f", cc=3),
                       psC[:, 2 * F:5 * F].rearrange("p (cc f) -> p cc f", cc=3),
                       pvec[:, r, 2, :].unsqueeze(1).to_broadcast([128, 3, F]),
                       op=AluOpType.mult)
                    TT(xf32[:, b, F:ROW], xf32[:, b, F:ROW], t3[:],
                       op=AluOpType.add)
                    TT(t3[:].rearrange("p (cc f) -> p cc f", cc=3),
                       psC[:, 5 * F:8 * F].rearrange("p (cc f) -> p cc f", cc=3),
                       pvec[:, r, 3, :].unsqueeze(1).to_broadcast([128, 3, F]),
                       op=AluOpType.mult)
                    TT(xf32[:, b, F:ROW], xf32[:, b, F:ROW], t3[:],
                       op=AluOpType.add)
                    TT(t3[:].rearrange("p (cc f) -> p cc f", cc=3),
                       psC[:, 8 * F:11 * F].rearrange("p (cc f) -> p cc f", cc=3),
                       pvec[:, r, 4, :].unsqueeze(1).to_broadcast([128, 3, F]),
                       op=AluOpType.mult)
                    TT(xf32[:, b, F:ROW], xf32[:, b, F:ROW], t3[:],
                       op=AluOpType.add)

                    if r < c.R_REF - 1:
                        xbf = scr.tile([128, ROW], dt.bfloat16, tag="xbf")
                        nc.scalar.activation(
                            xbf[:], xf32[:, b, :],
                            mybir.ActivationFunctionType.Copy)
                        nc.sync.dma_start(
                            xslice_d[b * 128:(b + 1) * 128, :], xbf[:])
                if r < c.R_REF - 1:
                    nc.gpsimd.collective_compute(
                        "AllGather",
                        mybir.AluOpType.bypass,
                        replica_groups=[list(range(c.N_CORES))],
                        ins=[xslice_d[:]],
                        outs=[tabs[r + 1][:]],
                    )

            for b in range(c.NBLK):
                tq = scr.tile([128, ROW], dt.float32, tag="tq")
                TT(tq[:], xf32[:, b, :], woutr[:], op=AluOpType.mult)
                nc.vector.tensor_reduce(
                    outsb[:, b, :].unsqueeze(-1),
                    tq[:].rearrange("p (l f) -> p l f", l=4),
                    axis=mybir.AxisListType.X,
                    op=AluOpType.add,
                )
            nc.sync.dma_start(out_d[:].rearrange("b p l -> p b l"), outsb[:])

    nc.compile()
    _BUILD_CACHE[key] = nc
    return nc


# ---------------------------------------------------------------- runner
def prep_in_maps(inputs):
    import ml_dtypes
    c = CFG
    bf16 = ml_dtypes.bfloat16

    x_dftb = np.asarray(inputs["x_dftb"], np.float32)
    coords = np.asarray(inputs["coords"], np.float32)
    dst_idx = np.asarray(inputs["dst_idx"], np.int32)
    src_idx = np.asarray(inputs["src_idx"], np.int32)
    W_in0 = np.asarray(inputs["W_in0"], np.float32)
    W_in1 = np.asarray(inputs["W_in1"], np.float32)
    b_in = np.asarray(inputs["b_in"], np.float32)
    W_basis = np.asarray(inputs["W_basis"], np.float32)
    b_basis = np.asarray(inputs["b_basis"], np.float32)
    path_w = np.asarray(inputs["path_w"], np.float32)
    W_out0 = np.asarray(inputs["W_out0"], np.float32)
    W_out1 = np.asarray(inputs["W_out1"], np.float32)

    rhat, rbf = _host_geometry(coords, dst_idx, src_idx)
    Wx = _host_weights(W_basis)

    x0 = _tensor_dense_np(x_dftb, W_in0, W_in1, b_in)
    x0_rows = x0.reshape(c.N_NODES, c.ROW)
    x0_rows_bf = x0_rows.astype(bf16)

    shards, slot_all = _host_shard(dst_idx, src_idx, rhat, rbf, x0_rows_bf)

    wout = np.concatenate(
        [W_out0[:, 0], W_out1[:, 0], W_out1[:, 0], W_out1[:, 0]]
    ).astype(np.float32)
    wout_rep = np.ascontiguousarray(
        np.broadcast_to(wout[None, :], (128, c.ROW)))

    wext_bf = Wx.astype(bf16)
    # path weights broadcast to 128 partitions: [R, 5, 128, F]
    pvec = np.ascontiguousarray(
        np.broadcast_to(path_w[:, :, None, :], (c.R_REF, 5, 128, c.F))
    ).astype(bf16)
    bvec = np.ascontiguousarray(
        np.broadcast_to(b_basis[:, None, :], (c.R_REF, 128, c.F))
    ).astype(bf16)

    in_maps = []
    for ci in range(c.N_CORES):
        sh = shards[ci]
        slot = slot_all[ci * c.NPC:(ci + 1) * c.NPC]
        x0c = np.zeros((c.NLOC, c.ROW), np.float32)
        x0c[slot] = x0_rows[ci * c.NPC:(ci + 1) * c.NPC]
        in_maps.append(dict(
            x0f=np.ascontiguousarray(x0c.reshape(c.NBLK, 128, c.ROW)),
            a0=np.ascontiguousarray(
                sh["a0"].reshape(c.NBLK, 128, c.TPB * c.ROW)),
            srcw=sh["srcw"],
            smat=sh["smat"].astype(bf16),
            basisT=sh["basisT"].astype(bf16),
            wext=wext_bf,
            pvec=pvec,
            bvec=bvec,
            woutrep=wout_rep,
        ))
    with_bias = bool(np.any(b_basis != 0.0))
    return in_maps, with_bias, slot_all


def assemble_output(results, b_out, slot_all):
    c = CFG
    outs = []
    for ci in range(c.N_CORES):
        o = np.asarray(results[ci]["out"]).reshape(c.NLOC, 4)
        slot = slot_all[ci * c.NPC:(ci + 1) * c.NPC]
        outs.append(o[slot])
    out = np.concatenate(outs, axis=0).reshape(c.N_NODES, 4, 1)
    out = out.astype(np.float32).copy()
    out[:, 0, :] += np.asarray(b_out, np.float32)[None, :]
    return out


LAST_RESULTS = None


def kernel(**inputs):
    global LAST_RESULTS
    from concourse.bass_utils import run_bass_kernel_spmd

    in_maps, with_bias, slot_all = prep_in_maps(inputs)
    nc = build(with_bias)
    trace = bool(os.environ.get("KERNEL_TRACE"))
    res = run_bass_kernel_spmd(
        nc, in_maps, core_ids=list(range(CFG.N_CORES)), trace=trace)
    LAST_RESULTS = res
    return assemble_output(res.results, inputs["b_out"], slot_all)
